# revision 18
# baseline (speedup 1.0000x reference)
"""NeuralAdditiveModel TRN2 kernel.

out[b] = sum_f ( relu(relu(x[b,f]*W1[f,:]+b1[f,:]) @ W2[f] + b2[f]) @ W3[f] + b3[f] ) + bias

Sharding: data-parallel over batch, 8 cores x 1024 rows. No collectives.

Per-core dataflow: 64 groups, each 4 features x 512-batch chunk, software
pipelined. The PE on this part streams at ~1.2GHz, so phase count per group
is what matters; it runs 3.5 phases of ~512 cycles per group:
  z1 : four K=2 matmuls row-tiled on all 4 strips (one phase) -> pz1a/pz1b
  z2 : two col-tiled M=64 matmuls per slot (two phases)       -> pz2a/pz2b
  z3 : every other group, FOUR M=1 matmuls on col strips 0..3, concurrent
       (one phase per two groups) -> pout rows 0/32/64/96
Relu drains are fixed-assigned: DVE h1a+h2b, ACT h1b+h2a (~balanced for the
1.2 vs 0.96 GHz engine rates).

Group G=(bt,g) covers feats {g, g+64} (slot a) and {g+32, g+96} (slot b);
issue order per G: z1(G+1), z2(G), h1(G+1), z3quad(G-2,G-1), h2(G).
PSUM: pz1 4 banks + pz2 2 + pout 1 = 7 of 8.
"""

import sys
from contextlib import ExitStack

import numpy as np

sys.path.insert(0, "/opt/trn_rl_repo")

import concourse.bass as bass  # noqa: E402
import concourse.tile as tile  # noqa: E402
from concourse import bacc, mybir  # noqa: E402
from concourse.bass_utils import run_bass_kernel_spmd  # noqa: E402

B, F, S, H1 = 8192, 128, 128, 64
NCORES = 8
BLOC = B // NCORES   # 1024 rows per core
BT = 512             # batch chunk (PSUM bank width in fp32)
NBT = BLOC // BT     # 2
NG = 32              # feature groups per chunk
NGRP = NBT * NG      # 64 pipeline groups
F32 = mybir.dt.float32
BF16 = mybir.dt.bfloat16

_CACHE = {}


def _build():
    nc = bacc.Bacc(
        "TRN2",
        target_bir_lowering=False,
        debug=False,
        enable_asserts=False,
        num_devices=NCORES,
    )

    xg_d = nc.dram_tensor("xg", [4, 32 * BLOC], BF16, kind="ExternalInput").ap()
    ones_d = nc.dram_tensor("ones", [1, 32 * BLOC], BF16, kind="ExternalInput").ap()
    w1q_d = nc.dram_tensor("w1q", [4, 32 * S], BF16, kind="ExternalInput").ap()
    b1q_d = nc.dram_tensor("b1q", [4, 32 * S], BF16, kind="ExternalInput").ap()
    w2t_d = nc.dram_tensor("w2t", [S, F * H1], BF16, kind="ExternalInput").ap()
    b2p_d = nc.dram_tensor("b2p", [2 * H1, F // 2], F32, kind="ExternalInput").ap()
    w3p_d = nc.dram_tensor("w3p", [2 * H1, F // 2], BF16, kind="ExternalInput").ap()
    out_d = nc.dram_tensor("out", [NBT * 4, BT], F32, kind="ExternalOutput").ap()

    Relu = mybir.ActivationFunctionType.Relu
    Copy = mybir.ActivationFunctionType.Copy

    with tile.TileContext(nc) as tc, ExitStack() as ctx:
        singles = ctx.enter_context(tc.tile_pool(name="singles", bufs=1))
        h1_pool = ctx.enter_context(tc.tile_pool(name="h1p", bufs=3))
        h2_pool = ctx.enter_context(tc.tile_pool(name="h2p", bufs=5))
        ps = ctx.enter_context(tc.tile_pool(name="ps", bufs=1, space="PSUM"))

        # Persistent SBUF tensors
        xaug = singles.tile([128, 32 * BLOC], BF16)  # x rows (32i) + ones (32i+1)
        w1b1 = singles.tile([128, 32 * S], BF16)     # W1 rows (32i) + b1 (32i+1)
        w2sb = singles.tile([S, F * H1], BF16)       # W2, s-major
        b2p = singles.tile([2 * H1, F // 2], F32)    # paired bias columns
        w3p = singles.tile([2 * H1, F // 2], BF16)   # paired W3 columns

        # Setup DMAs spread across the 3 DMA-capable queues, ordered by first
        # use: all 16 z1 rows first (single-partition rows are the slow
        # transfers), then the w2 quarters, then small bias tables.
        FQ = F // 4 * H1  # w2t column quarter
        qs = (nc.sync, nc.scalar, nc.gpsimd)

        for i in range(4):
            qs[i % 3].dma_start(
                out=w1b1[32 * i : 32 * i + 1, :], in_=w1q_d[i : i + 1, :]
            )
            qs[(i + 1) % 3].dma_start(
                out=w1b1[32 * i + 1 : 32 * i + 2, :], in_=b1q_d[i : i + 1, :]
            )
        CW = 32 * BLOC // 4  # x/ones row quarter: 8 feature-blocks
        for c in range(4):
            cl, ch = c * CW, (c + 1) * CW
            for i in range(4):
                q = (i + c) % 3
                qs[q].dma_start(
                    out=xaug[32 * i : 32 * i + 1, cl:ch], in_=xg_d[i : i + 1, cl:ch]
                )
                qs[(q + 1) % 3].dma_start(
                    out=xaug[32 * i + 1 : 32 * i + 2, cl:ch], in_=ones_d[0:1, cl:ch]
                )
            if c == 0:
                # w2 quarters needed by the first z2s go right after chunk 0
                qs[0].dma_start(out=w2sb[:, 0:FQ], in_=w2t_d[:, 0:FQ])
                qs[1].dma_start(
                    out=w2sb[:, 2 * FQ : 3 * FQ], in_=w2t_d[:, 2 * FQ : 3 * FQ]
                )
                qs[2].dma_start(out=b2p, in_=b2p_d)
                qs[2].dma_start(out=w3p, in_=w3p_d)
            if c == 1:
                qs[0].dma_start(out=w2sb[:, FQ : 2 * FQ], in_=w2t_d[:, FQ : 2 * FQ])
                qs[1].dma_start(out=w2sb[:, 3 * FQ :], in_=w2t_d[:, 3 * FQ :])

        def grp(G):  # group -> (bt, g)
            return G // NG, G % NG

        def z1(G, pza, pzb):
            bt, g = grp(G)
            # strips q0,q32,q64,q96 <-> feats g, g+32, g+64, g+96; 4 banks
            for i, pz, half in ((0, pza, 0), (1, pzb, 0), (2, pza, 1), (3, pzb, 1)):
                r = 32 * i
                nc.tensor.matmul(
                    out=pz[:, half * BT : (half + 1) * BT],
                    lhsT=w1b1[r : r + 2, g * S : (g + 1) * S],
                    rhs=xaug[r : r + 2, g * BLOC + bt * BT : g * BLOC + (bt + 1) * BT],
                    start=True,
                    stop=True,
                    tile_position=(r, 0),
                )

        def z2half(G, sub, half, h1sb, pz2):
            # one M=64 matmul: half=0 -> out rows 0-63 (col strips 0-1),
            # half=1 -> rows 64-127 (strips 2-3)
            _, g = grp(G)
            j = g + 32 * sub
            f = j + 64 * half
            nc.tensor.matmul(
                out=pz2[64 * half : 64 * half + 64, :],
                lhsT=w2sb[:, f * H1 : (f + 1) * H1],
                rhs=h1sb[:, half * BT : (half + 1) * BT],
                start=True,
                stop=True,
            )

        def h1drain(G, sub, pz, h1sb):
            if sub == 0:
                nc.vector.tensor_scalar_max(h1sb, pz, 0.0)
            else:
                nc.scalar.activation(h1sb, pz, Relu)

        def h2drain(G, sub, pz2, h2sb):
            _, g = grp(G)
            j = g + 32 * sub
            if sub == 0:
                nc.scalar.activation(h2sb, pz2, Relu, bias=b2p[:, j : j + 1])
            else:
                nc.vector.tensor_scalar(
                    h2sb,
                    pz2,
                    b2p[:, j : j + 1],
                    0.0,
                    mybir.AluOpType.add,
                    mybir.AluOpType.max,
                )

        def z3(q, sub, h2sb, pout):
            bt, g = grp(q)
            j = g + 32 * sub
            row = 32 * (2 * (q % 2) + sub)  # col strips 0..3 across the quad
            nc.tensor.matmul(
                out=pout[row : row + 1, :],
                lhsT=w3p[:, j : j + 1],
                rhs=h2sb,
                start=(g <= 1),
                stop=(g >= NG - 2),
                skip_group_check=True,
                tile_position=(0, row),
            )

        def pout_flush(bt, pout):
            srow = h2_pool.tile([128, BT], F32, tag="srow", name="srow")
            nc.scalar.activation(srow[0:97, :], pout[0:97, :], Copy)
            srow_g = srow.rearrange("(i q) c -> i q c", q=32)
            nc.sync.dma_start(out=out_d[4 * bt : 4 * bt + 4, :], in_=srow_g[:, 0, :])

        pz1a_t = [None] * NGRP
        pz1b_t = [None] * NGRP
        h1a_t = [None] * NGRP
        h1b_t = [None] * NGRP
        pz2a_t = [None] * NGRP
        pz2b_t = [None] * NGRP
        h2a_t = [None] * NGRP
        h2b_t = [None] * NGRP
        pout_t = [None] * NBT

        def alloc_z1(G):
            pz1a_t[G] = ps.tile([128, 2 * BT], F32, tag="pz1a", name="pz1a")
            pz1b_t[G] = ps.tile([128, 2 * BT], F32, tag="pz1b", name="pz1b")

        def alloc_h1(G):
            h1a_t[G] = h1_pool.tile([128, 2 * BT], BF16, tag="h1a", name="h1a")
            h1b_t[G] = h1_pool.tile([128, 2 * BT], BF16, tag="h1b", name="h1b")

        def z3pair(q):
            # one z3 pair; strips 0-1 for even q, 2-3 for odd q, so the
            # opposite-strip z2 halves issued around it can overlap
            bt, g = grp(q)
            if g == 0:
                pout_t[bt] = ps.tile([128, BT], F32, tag="pout", name="pout")
            z3(q, 0, h2a_t[q], pout_t[bt])
            z3(q, 1, h2b_t[q], pout_t[bt])
            if g == NG - 1:
                pout_flush(bt, pout_t[bt])

        alloc_z1(0)
        z1(0, pz1a_t[0], pz1b_t[0])
        alloc_h1(0)
        h1drain(0, 0, pz1a_t[0], h1a_t[0])
        h1drain(0, 1, pz1b_t[0], h1b_t[0])

        for G in range(NGRP):
            par = G % 2
            pz2a_t[G] = ps.tile([128, BT], F32, tag="pz2a", name="pz2a")
            pz2b_t[G] = ps.tile([128, BT], F32, tag="pz2b", name="pz2b")
            if G >= 2:
                z3pair(G - 2)  # strips 0-1 (even G) / 2-3 (odd G)
            # opposite-strip z2a half rides alongside the z3 pair
            z2half(G, 0, 1 - par, h1a_t[G], pz2a_t[G])
            if G + 1 < NGRP:
                alloc_z1(G + 1)
                z1(G + 1, pz1a_t[G + 1], pz1b_t[G + 1])
            # remaining z2 halves, alternating strips so adjacent MMs pack
            z2half(G, 0, par, h1a_t[G], pz2a_t[G])
            z2half(G, 1, 1 - par, h1b_t[G], pz2b_t[G])
            z2half(G, 1, par, h1b_t[G], pz2b_t[G])
            if G + 1 < NGRP:
                alloc_h1(G + 1)
                h1drain(G + 1, 0, pz1a_t[G + 1], h1a_t[G + 1])
                h1drain(G + 1, 1, pz1b_t[G + 1], h1b_t[G + 1])
            h2a_t[G] = h2_pool.tile([128, BT], BF16, tag="h2a", name="h2a")
            h2drain(G, 0, pz2a_t[G], h2a_t[G])
            h2b_t[G] = h2_pool.tile([128, BT], BF16, tag="h2b", name="h2b")
            h2drain(G, 1, pz2b_t[G], h2b_t[G])

        z3pair(NGRP - 2)
        z3pair(NGRP - 1)

    nc.compile()
    return nc


def _prep_shared(W1, b1, W2, b2, W3):
    import ml_dtypes

    bf = ml_dtypes.bfloat16
    w1q = np.ascontiguousarray(W1.reshape(4, 32 * S)).astype(bf)
    b1q = np.ascontiguousarray(b1.reshape(4, 32 * S)).astype(bf)
    w2t = np.ascontiguousarray(W2.transpose(1, 0, 2).reshape(S, F * H1)).astype(bf)
    b2p = np.empty((2 * H1, F // 2), np.float32)
    w3p = np.empty((2 * H1, F // 2), np.float32)
    W3f = W3.reshape(F, H1)
    for j in range(F // 2):
        b2p[:H1, j] = b2[j]
        b2p[H1:, j] = b2[j + 64]
        w3p[:H1, j] = W3f[j]
        w3p[H1:, j] = W3f[j + 64]
    return {
        "w1q": w1q,
        "b1q": b1q,
        "w2t": w2t,
        "b2p": b2p,
        "w3p": w3p.astype(bf),
        "ones": np.ones((1, 32 * BLOC), bf),
    }


def _prep_core_inputs(xc, shared):
    import ml_dtypes

    m = dict(shared)
    # xg[i, g*BLOC + b] = x[b, 32i+g]
    m["xg"] = (
        np.ascontiguousarray(xc.T.reshape(4, 32 * BLOC)).astype(ml_dtypes.bfloat16)
    )
    return m


def kernel(x, W1, b1, W2, b2, W3, b3, bias, _trace=False):
    x = np.asarray(x, np.float32)
    W1 = np.asarray(W1, np.float32)
    b1 = np.asarray(b1, np.float32)
    W2 = np.asarray(W2, np.float32)
    b2 = np.asarray(b2, np.float32)
    W3 = np.asarray(W3, np.float32)
    b3 = np.asarray(b3, np.float32)
    bias = np.asarray(bias, np.float32)

    if "nc" not in _CACHE:
        _CACHE["nc"] = _build()
    nc = _CACHE["nc"]

    shared = _prep_shared(W1, b1, W2, b2, W3)
    in_maps = [
        _prep_core_inputs(x[c * BLOC : (c + 1) * BLOC], shared) for c in range(NCORES)
    ]

    res = run_bass_kernel_spmd(nc, in_maps, core_ids=list(range(NCORES)), trace=_trace)
    _CACHE["last_result"] = res

    const = float(b3.sum()) + float(bias.reshape(-1)[0])
    parts = []
    for c in range(NCORES):
        o = res.results[c]["out"]  # [NBT*4, BT]: pout rows 0/32/64/96 per chunk
        parts.append(o.reshape(NBT, 4, BT).sum(axis=1).reshape(BLOC))
    out = np.concatenate(parts) + const
    return out.reshape(B, 1).astype(np.float32)


# revision 19
# speedup vs baseline: 1.0329x; 1.0329x over previous
"""NeuralAdditiveModel TRN2 kernel.

out[b] = sum_f ( relu(relu(x[b,f]*W1[f,:]+b1[f,:]) @ W2[f] + b2[f]) @ W3[f] + b3[f] ) + bias

Sharding: data-parallel over batch, 8 cores x 1024 rows. No collectives.

Per-core dataflow: 64 groups, each 4 features x 512-batch chunk, software
pipelined. The PE on this part streams at ~1.2GHz, so phase count per group
is what matters; it runs 3.5 phases of ~512 cycles per group:
  z1 : four K=2 matmuls row-tiled on all 4 strips (one phase) -> pz1a/pz1b
  z2 : two col-tiled M=64 matmuls per slot (two phases)       -> pz2a/pz2b
  z3 : every other group, FOUR M=1 matmuls on col strips 0..3, concurrent
       (one phase per two groups) -> pout rows 0/32/64/96
Relu drains are fixed-assigned: DVE h1a+h2b, ACT h1b+h2a (~balanced for the
1.2 vs 0.96 GHz engine rates).

Group G=(bt,g) covers feats {g, g+64} (slot a) and {g+32, g+96} (slot b);
issue order per G: z1(G+1), z2(G), h1(G+1), z3quad(G-2,G-1), h2(G).
PSUM: pz1 4 banks + pz2 2 + pout 1 = 7 of 8.
"""

import sys
from contextlib import ExitStack

import numpy as np

sys.path.insert(0, "/opt/trn_rl_repo")

import concourse.bass as bass  # noqa: E402
import concourse.tile as tile  # noqa: E402
from concourse import bacc, mybir  # noqa: E402
from concourse.bass_utils import run_bass_kernel_spmd  # noqa: E402

B, F, S, H1 = 8192, 128, 128, 64
NCORES = 8
BLOC = B // NCORES   # 1024 rows per core
BT = 512             # batch chunk (PSUM bank width in fp32)
NBT = BLOC // BT     # 2
NG = 32              # feature groups per chunk
NGRP = NBT * NG      # 64 pipeline groups
F32 = mybir.dt.float32
BF16 = mybir.dt.bfloat16

_CACHE = {}


def _build():
    nc = bacc.Bacc(
        "TRN2",
        target_bir_lowering=False,
        debug=False,
        enable_asserts=False,
        num_devices=NCORES,
    )

    xg_d = nc.dram_tensor("xg", [4, 32 * BLOC], BF16, kind="ExternalInput").ap()
    ones_d = nc.dram_tensor("ones", [1, 32 * BLOC], BF16, kind="ExternalInput").ap()
    w1q_d = nc.dram_tensor("w1q", [4, 32 * S], BF16, kind="ExternalInput").ap()
    b1q_d = nc.dram_tensor("b1q", [4, 32 * S], BF16, kind="ExternalInput").ap()
    w2t_d = nc.dram_tensor("w2t", [S, F * H1], BF16, kind="ExternalInput").ap()
    b2p_d = nc.dram_tensor("b2p", [2 * H1, F // 2], F32, kind="ExternalInput").ap()
    w3p_d = nc.dram_tensor("w3p", [2 * H1, F // 2], BF16, kind="ExternalInput").ap()
    out_d = nc.dram_tensor("out", [NBT * 4, BT], F32, kind="ExternalOutput").ap()

    Relu = mybir.ActivationFunctionType.Relu
    Copy = mybir.ActivationFunctionType.Copy

    with tile.TileContext(nc) as tc, ExitStack() as ctx:
        singles = ctx.enter_context(tc.tile_pool(name="singles", bufs=1))
        h1_pool = ctx.enter_context(tc.tile_pool(name="h1p", bufs=3))
        h2_pool = ctx.enter_context(tc.tile_pool(name="h2p", bufs=5))
        ps = ctx.enter_context(tc.tile_pool(name="ps", bufs=1, space="PSUM"))

        # Persistent SBUF tensors
        xaug = singles.tile([128, 32 * BLOC], BF16)  # x rows (32i) + ones (32i+1)
        w1b1 = singles.tile([128, 32 * S], BF16)     # W1 rows (32i) + b1 (32i+1)
        w2sb = singles.tile([S, F * H1], BF16)       # W2, s-major
        b2p = singles.tile([2 * H1, F // 2], F32)    # paired bias columns
        w3p = singles.tile([2 * H1, F // 2], BF16)   # paired W3 columns

        # Setup DMAs spread across the 3 DMA-capable queues, ordered by first
        # use: all 16 z1 rows first (single-partition rows are the slow
        # transfers), then the w2 quarters, then small bias tables.
        FQ = F // 4 * H1  # w2t column quarter
        qs = (nc.sync, nc.scalar, nc.gpsimd)

        for i in range(4):
            qs[i % 3].dma_start(
                out=w1b1[32 * i : 32 * i + 1, :], in_=w1q_d[i : i + 1, :]
            )
            qs[(i + 1) % 3].dma_start(
                out=w1b1[32 * i + 1 : 32 * i + 2, :], in_=b1q_d[i : i + 1, :]
            )
        CW = 32 * BLOC // 4  # x/ones row quarter: 8 feature-blocks
        for c in range(4):
            cl, ch = c * CW, (c + 1) * CW
            for i in range(4):
                q = (i + c) % 3
                qs[q].dma_start(
                    out=xaug[32 * i : 32 * i + 1, cl:ch], in_=xg_d[i : i + 1, cl:ch]
                )
                qs[(q + 1) % 3].dma_start(
                    out=xaug[32 * i + 1 : 32 * i + 2, cl:ch], in_=ones_d[0:1, cl:ch]
                )
            if c == 0:
                # w2 quarters needed by the first z2s go right after chunk 0
                qs[0].dma_start(out=w2sb[:, 0:FQ], in_=w2t_d[:, 0:FQ])
                qs[1].dma_start(
                    out=w2sb[:, 2 * FQ : 3 * FQ], in_=w2t_d[:, 2 * FQ : 3 * FQ]
                )
                qs[2].dma_start(out=b2p, in_=b2p_d)
                qs[2].dma_start(out=w3p, in_=w3p_d)
            if c == 1:
                qs[0].dma_start(out=w2sb[:, FQ : 2 * FQ], in_=w2t_d[:, FQ : 2 * FQ])
                qs[1].dma_start(out=w2sb[:, 3 * FQ :], in_=w2t_d[:, 3 * FQ :])

        def grp(G):  # group -> (bt, g)
            return G // NG, G % NG

        def z1(G, pza, pzb):
            bt, g = grp(G)
            # strips q0,q32,q64,q96 <-> feats g, g+32, g+64, g+96; 4 banks
            for i, pz, half in ((0, pza, 0), (1, pzb, 0), (2, pza, 1), (3, pzb, 1)):
                r = 32 * i
                nc.tensor.matmul(
                    out=pz[:, half * BT : (half + 1) * BT],
                    lhsT=w1b1[r : r + 2, g * S : (g + 1) * S],
                    rhs=xaug[r : r + 2, g * BLOC + bt * BT : g * BLOC + (bt + 1) * BT],
                    start=True,
                    stop=True,
                    tile_position=(r, 0),
                )

        def z2half(G, sub, half, h1sb, pz2):
            # one M=64 matmul: half=0 -> out rows 0-63 (col strips 0-1),
            # half=1 -> rows 64-127 (strips 2-3)
            _, g = grp(G)
            j = g + 32 * sub
            f = j + 64 * half
            nc.tensor.matmul(
                out=pz2[64 * half : 64 * half + 64, :],
                lhsT=w2sb[:, f * H1 : (f + 1) * H1],
                rhs=h1sb[:, half * BT : (half + 1) * BT],
                start=True,
                stop=True,
            )

        def h1drain(G, sub, pz, h1sb):
            if sub == 0:
                nc.vector.tensor_scalar_max(h1sb, pz, 0.0)
            else:
                nc.scalar.activation(h1sb, pz, Relu)

        def h2drain(G, sub, pz2, h2sb):
            _, g = grp(G)
            j = g + 32 * sub
            if sub == 0:
                nc.scalar.activation(h2sb, pz2, Relu, bias=b2p[:, j : j + 1])
            else:
                nc.vector.tensor_scalar(
                    h2sb,
                    pz2,
                    b2p[:, j : j + 1],
                    0.0,
                    mybir.AluOpType.add,
                    mybir.AluOpType.max,
                )

        def z3(q, sub, h2sb, pout):
            bt, g = grp(q)
            j = g + 32 * sub
            row = 32 * (2 * (q % 2) + sub)  # col strips 0..3 across the quad
            nc.tensor.matmul(
                out=pout[row : row + 1, :],
                lhsT=w3p[:, j : j + 1],
                rhs=h2sb,
                start=(g <= 1),
                stop=(g >= NG - 2),
                skip_group_check=True,
                tile_position=(0, row),
            )

        def pout_flush(bt, pout):
            srow = h2_pool.tile([128, BT], F32, tag="srow", name="srow")
            nc.scalar.activation(srow[0:97, :], pout[0:97, :], Copy)
            srow_g = srow.rearrange("(i q) c -> i q c", q=32)
            nc.sync.dma_start(out=out_d[4 * bt : 4 * bt + 4, :], in_=srow_g[:, 0, :])

        pz1a_t = [None] * NGRP
        pz1b_t = [None] * NGRP
        h1a_t = [None] * NGRP
        h1b_t = [None] * NGRP
        pz2a_t = [None] * NGRP
        pz2b_t = [None] * NGRP
        h2a_t = [None] * NGRP
        h2b_t = [None] * NGRP
        pout_t = [None] * NBT

        def alloc_z1(G):
            pz1a_t[G] = ps.tile([128, 2 * BT], F32, tag="pz1a", name="pz1a")
            pz1b_t[G] = ps.tile([128, 2 * BT], F32, tag="pz1b", name="pz1b")

        def alloc_h1(G):
            h1a_t[G] = h1_pool.tile([128, 2 * BT], BF16, tag="h1a", name="h1a")
            h1b_t[G] = h1_pool.tile([128, 2 * BT], BF16, tag="h1b", name="h1b")

        def z3pair(q):
            # one z3 pair; strips 0-1 for even q, 2-3 for odd q, so the
            # opposite-strip z2 halves issued around it can overlap
            bt, g = grp(q)
            if g == 0:
                pout_t[bt] = ps.tile([128, BT], F32, tag="pout", name="pout")
            z3(q, 0, h2a_t[q], pout_t[bt])
            z3(q, 1, h2b_t[q], pout_t[bt])
            if g == NG - 1:
                pout_flush(bt, pout_t[bt])

        alloc_z1(0)
        z1(0, pz1a_t[0], pz1b_t[0])
        alloc_h1(0)
        h1drain(0, 0, pz1a_t[0], h1a_t[0])
        h1drain(0, 1, pz1b_t[0], h1b_t[0])

        for G in range(NGRP):
            par = G % 2
            pz2a_t[G] = ps.tile([128, BT], F32, tag="pz2a", name="pz2a")
            pz2b_t[G] = ps.tile([128, BT], F32, tag="pz2b", name="pz2b")
            if G >= 2:
                z3pair(G - 2)  # strips 0-1 (even G) / 2-3 (odd G); rides the
                # previous slot's tail z2 half (opposite strips, stale deps)
            if G + 1 < NGRP:
                alloc_z1(G + 1)
                z1(G + 1, pz1a_t[G + 1], pz1b_t[G + 1])
            # z2 halves alternate strips so adjacent MMs pack pairwise and the
            # last half (par strips) is disjoint from the next slot's z3 pair
            z2half(G, 0, 1 - par, h1a_t[G], pz2a_t[G])
            z2half(G, 0, par, h1a_t[G], pz2a_t[G])
            z2half(G, 1, 1 - par, h1b_t[G], pz2b_t[G])
            z2half(G, 1, par, h1b_t[G], pz2b_t[G])
            if G + 1 < NGRP:
                alloc_h1(G + 1)
                h1drain(G + 1, 0, pz1a_t[G + 1], h1a_t[G + 1])
                h1drain(G + 1, 1, pz1b_t[G + 1], h1b_t[G + 1])
            h2a_t[G] = h2_pool.tile([128, BT], BF16, tag="h2a", name="h2a")
            h2drain(G, 0, pz2a_t[G], h2a_t[G])
            h2b_t[G] = h2_pool.tile([128, BT], BF16, tag="h2b", name="h2b")
            h2drain(G, 1, pz2b_t[G], h2b_t[G])

        z3pair(NGRP - 2)
        z3pair(NGRP - 1)

    nc.compile()
    return nc


def _prep_shared(W1, b1, W2, b2, W3):
    import ml_dtypes

    bf = ml_dtypes.bfloat16
    w1q = np.ascontiguousarray(W1.reshape(4, 32 * S)).astype(bf)
    b1q = np.ascontiguousarray(b1.reshape(4, 32 * S)).astype(bf)
    w2t = np.ascontiguousarray(W2.transpose(1, 0, 2).reshape(S, F * H1)).astype(bf)
    b2p = np.empty((2 * H1, F // 2), np.float32)
    w3p = np.empty((2 * H1, F // 2), np.float32)
    W3f = W3.reshape(F, H1)
    for j in range(F // 2):
        b2p[:H1, j] = b2[j]
        b2p[H1:, j] = b2[j + 64]
        w3p[:H1, j] = W3f[j]
        w3p[H1:, j] = W3f[j + 64]
    return {
        "w1q": w1q,
        "b1q": b1q,
        "w2t": w2t,
        "b2p": b2p,
        "w3p": w3p.astype(bf),
        "ones": np.ones((1, 32 * BLOC), bf),
    }


def _prep_core_inputs(xc, shared):
    import ml_dtypes

    m = dict(shared)
    # xg[i, g*BLOC + b] = x[b, 32i+g]
    m["xg"] = (
        np.ascontiguousarray(xc.T.reshape(4, 32 * BLOC)).astype(ml_dtypes.bfloat16)
    )
    return m


def kernel(x, W1, b1, W2, b2, W3, b3, bias, _trace=False):
    x = np.asarray(x, np.float32)
    W1 = np.asarray(W1, np.float32)
    b1 = np.asarray(b1, np.float32)
    W2 = np.asarray(W2, np.float32)
    b2 = np.asarray(b2, np.float32)
    W3 = np.asarray(W3, np.float32)
    b3 = np.asarray(b3, np.float32)
    bias = np.asarray(bias, np.float32)

    if "nc" not in _CACHE:
        _CACHE["nc"] = _build()
    nc = _CACHE["nc"]

    shared = _prep_shared(W1, b1, W2, b2, W3)
    in_maps = [
        _prep_core_inputs(x[c * BLOC : (c + 1) * BLOC], shared) for c in range(NCORES)
    ]

    res = run_bass_kernel_spmd(nc, in_maps, core_ids=list(range(NCORES)), trace=_trace)
    _CACHE["last_result"] = res

    const = float(b3.sum()) + float(bias.reshape(-1)[0])
    parts = []
    for c in range(NCORES):
        o = res.results[c]["out"]  # [NBT*4, BT]: pout rows 0/32/64/96 per chunk
        parts.append(o.reshape(NBT, 4, BT).sum(axis=1).reshape(BLOC))
    out = np.concatenate(parts) + const
    return out.reshape(B, 1).astype(np.float32)


# revision 21
# speedup vs baseline: 1.2232x; 1.1843x over previous
"""NeuralAdditiveModel TRN2 kernel.

out[b] = sum_f ( relu(relu(x[b,f]*W1[f,:]+b1[f,:]) @ W2[f] + b2[f]) @ W3[f] + b3[f] ) + bias

Sharding: data-parallel over batch, 8 cores x 1024 rows. No collectives.

Per-core dataflow: 64 groups, each 4 features x 512-batch chunk, software
pipelined. The PE on this part streams at ~1.2GHz, so phase count per group
is what matters; it runs 3.5 phases of ~512 cycles per group:
  z1 : four K=2 matmuls row-tiled on all 4 strips (one phase) -> pz1a/pz1b
  z2 : two col-tiled M=64 matmuls per slot (two phases)       -> pz2a/pz2b
  z3 : every other group, FOUR M=1 matmuls on col strips 0..3, concurrent
       (one phase per two groups) -> pout rows 0/32/64/96
Relu drains are fixed-assigned: DVE h1a+h2b, ACT h1b+h2a (~balanced for the
1.2 vs 0.96 GHz engine rates).

Group G=(bt,g) covers feats {g, g+64} (slot a) and {g+32, g+96} (slot b);
issue order per G: z1(G+1), z2(G), h1(G+1), z3quad(G-2,G-1), h2(G).
PSUM: pz1 4 banks + pz2 2 + pout 1 = 7 of 8.
"""

import sys
from contextlib import ExitStack

import numpy as np

sys.path.insert(0, "/opt/trn_rl_repo")

import concourse.bass as bass  # noqa: E402
import concourse.tile as tile  # noqa: E402
from concourse import bacc, mybir  # noqa: E402
from concourse.bass_utils import run_bass_kernel_spmd  # noqa: E402

B, F, S, H1 = 8192, 128, 128, 64
NCORES = 8
BLOC = B // NCORES   # 1024 rows per core
BT = 512             # batch chunk (PSUM bank width in fp32)
NBT = BLOC // BT     # 2
NG = 32              # feature groups per chunk
NGRP = NBT * NG      # 64 pipeline groups
F32 = mybir.dt.float32
BF16 = mybir.dt.bfloat16

_CACHE = {}


def _build():
    nc = bacc.Bacc(
        "TRN2",
        target_bir_lowering=False,
        debug=False,
        enable_asserts=False,
        num_devices=NCORES,
    )

    xg_d = nc.dram_tensor("xg", [4, 32 * BLOC], BF16, kind="ExternalInput").ap()
    ones_d = nc.dram_tensor("ones", [1, 32 * BLOC], BF16, kind="ExternalInput").ap()
    w1q_d = nc.dram_tensor("w1q", [4, 32 * S], BF16, kind="ExternalInput").ap()
    b1q_d = nc.dram_tensor("b1q", [4, 32 * S], BF16, kind="ExternalInput").ap()
    w2t_d = nc.dram_tensor("w2t", [S, F * H1], BF16, kind="ExternalInput").ap()
    b2p_d = nc.dram_tensor("b2p", [2 * H1, F // 2], F32, kind="ExternalInput").ap()
    w3p_d = nc.dram_tensor("w3p", [2 * H1, F // 2], BF16, kind="ExternalInput").ap()
    out_d = nc.dram_tensor("out", [NBT * 4, BT], F32, kind="ExternalOutput").ap()

    Relu = mybir.ActivationFunctionType.Relu
    Copy = mybir.ActivationFunctionType.Copy

    with tile.TileContext(nc) as tc, ExitStack() as ctx:
        singles = ctx.enter_context(tc.tile_pool(name="singles", bufs=1))
        h1_pool = ctx.enter_context(tc.tile_pool(name="h1p", bufs=4))
        h2_pool = ctx.enter_context(tc.tile_pool(name="h2p", bufs=5))
        ps = ctx.enter_context(tc.tile_pool(name="ps", bufs=1, space="PSUM"))

        # Persistent SBUF tensors
        xaug = singles.tile([128, 32 * BLOC], BF16)  # x rows (32i) + ones (32i+1)
        w1b1 = singles.tile([128, 32 * S], BF16)     # W1 rows (32i) + b1 (32i+1)
        w2sb = singles.tile([S, F * H1], BF16)       # W2, s-major
        b2p = singles.tile([2 * H1, F // 2], F32)    # paired bias columns
        w3p = singles.tile([2 * H1, F // 2], BF16)   # paired W3 columns

        # Setup DMAs spread across the 3 DMA-capable queues, ordered by first
        # use: all 16 z1 rows first (single-partition rows are the slow
        # transfers), then the w2 quarters, then small bias tables.
        FQ = F // 4 * H1  # w2t column quarter
        qs = (nc.sync, nc.scalar, nc.gpsimd)

        for i in range(4):
            qs[i % 3].dma_start(
                out=w1b1[32 * i : 32 * i + 1, :], in_=w1q_d[i : i + 1, :]
            )
            qs[(i + 1) % 3].dma_start(
                out=w1b1[32 * i + 1 : 32 * i + 2, :], in_=b1q_d[i : i + 1, :]
            )
        CW = 32 * BLOC // 4  # x/ones row quarter: 8 feature-blocks
        for c in range(4):
            cl, ch = c * CW, (c + 1) * CW
            for i in range(4):
                q = (i + c) % 3
                qs[q].dma_start(
                    out=xaug[32 * i : 32 * i + 1, cl:ch], in_=xg_d[i : i + 1, cl:ch]
                )
                qs[(q + 1) % 3].dma_start(
                    out=xaug[32 * i + 1 : 32 * i + 2, cl:ch], in_=ones_d[0:1, cl:ch]
                )
            if c == 0:
                # w2 quarters needed by the first z2s go right after chunk 0
                qs[0].dma_start(out=w2sb[:, 0:FQ], in_=w2t_d[:, 0:FQ])
                qs[1].dma_start(
                    out=w2sb[:, 2 * FQ : 3 * FQ], in_=w2t_d[:, 2 * FQ : 3 * FQ]
                )
                qs[2].dma_start(out=b2p, in_=b2p_d)
                qs[2].dma_start(out=w3p, in_=w3p_d)
            if c == 1:
                qs[0].dma_start(out=w2sb[:, FQ : 2 * FQ], in_=w2t_d[:, FQ : 2 * FQ])
                qs[1].dma_start(out=w2sb[:, 3 * FQ :], in_=w2t_d[:, 3 * FQ :])

        def grp(G):  # group -> (bt, g)
            return G // NG, G % NG

        def z1(G, pza, pzb):
            bt, g = grp(G)
            # strips q0,q32,q64,q96 <-> feats g, g+32, g+64, g+96; 4 banks
            for i, pz, half in ((0, pza, 0), (1, pzb, 0), (2, pza, 1), (3, pzb, 1)):
                r = 32 * i
                nc.tensor.matmul(
                    out=pz[:, half * BT : (half + 1) * BT],
                    lhsT=w1b1[r : r + 2, g * S : (g + 1) * S],
                    rhs=xaug[r : r + 2, g * BLOC + bt * BT : g * BLOC + (bt + 1) * BT],
                    start=True,
                    stop=True,
                    tile_position=(r, 0),
                )

        def z2(G, sub, h1sb, pz2):
            _, g = grp(G)
            j = g + 32 * sub
            for half, f in enumerate((j, j + 64)):
                nc.tensor.matmul(
                    out=pz2[64 * half : 64 * half + 64, :],
                    lhsT=w2sb[:, f * H1 : (f + 1) * H1],
                    rhs=h1sb[:, half * BT : (half + 1) * BT],
                    start=True,
                    stop=True,
                )

        def h1drain(G, sub, pz, h1sb):
            if sub == 0:
                nc.vector.tensor_scalar_max(h1sb, pz, 0.0)
            else:
                nc.scalar.activation(h1sb, pz, Relu)

        def h2drain(G, sub, pz2, h2sb):
            _, g = grp(G)
            j = g + 32 * sub
            if sub == 0:
                nc.scalar.activation(h2sb, pz2, Relu, bias=b2p[:, j : j + 1])
            else:
                nc.vector.tensor_scalar(
                    h2sb,
                    pz2,
                    b2p[:, j : j + 1],
                    0.0,
                    mybir.AluOpType.add,
                    mybir.AluOpType.max,
                )

        def z3(q, sub, h2sb, pout):
            bt, g = grp(q)
            j = g + 32 * sub
            row = 32 * (2 * (q % 2) + sub)  # col strips 0..3 across the quad
            nc.tensor.matmul(
                out=pout[row : row + 1, :],
                lhsT=w3p[:, j : j + 1],
                rhs=h2sb,
                start=(g <= 1),
                stop=(g >= NG - 2),
                skip_group_check=True,
                tile_position=(0, row),
            )

        def pout_flush(bt, pout):
            srow = h2_pool.tile([128, BT], F32, tag="srow", name="srow")
            nc.scalar.activation(srow[0:97, :], pout[0:97, :], Copy)
            srow_g = srow.rearrange("(i q) c -> i q c", q=32)
            nc.sync.dma_start(out=out_d[4 * bt : 4 * bt + 4, :], in_=srow_g[:, 0, :])

        pz1a_t = [None] * NGRP
        pz1b_t = [None] * NGRP
        h1a_t = [None] * NGRP
        h1b_t = [None] * NGRP
        pz2a_t = [None] * NGRP
        pz2b_t = [None] * NGRP
        h2a_t = [None] * NGRP
        h2b_t = [None] * NGRP
        pout_t = [None] * NBT

        def alloc_z1(G):
            pz1a_t[G] = ps.tile([128, 2 * BT], F32, tag="pz1a", name="pz1a")
            pz1b_t[G] = ps.tile([128, 2 * BT], F32, tag="pz1b", name="pz1b")

        def alloc_h1(G):
            h1a_t[G] = h1_pool.tile([128, 2 * BT], BF16, tag="h1a", name="h1a")
            h1b_t[G] = h1_pool.tile([128, 2 * BT], BF16, tag="h1b", name="h1b")

        def z3quad(G):
            # z3 for groups G-3, G-2 as four concurrent col-tiled matmuls
            # (1.5+ periods stale, so the quad never blocks the PE queue)
            for q in (G - 3, G - 2):
                bt, g = grp(q)
                if g == 0:
                    pout_t[bt] = ps.tile([128, BT], F32, tag="pout", name="pout")
                z3(q, 0, h2a_t[q], pout_t[bt])
                z3(q, 1, h2b_t[q], pout_t[bt])
                if g == NG - 1:
                    pout_flush(bt, pout_t[bt])

        alloc_z1(0)
        z1(0, pz1a_t[0], pz1b_t[0])
        alloc_h1(0)
        h1drain(0, 0, pz1a_t[0], h1a_t[0])
        h1drain(0, 1, pz1b_t[0], h1b_t[0])

        for G in range(NGRP):
            if G >= 3 and G % 2 == 1:
                z3quad(G)
            if G + 1 < NGRP:
                alloc_z1(G + 1)
                z1(G + 1, pz1a_t[G + 1], pz1b_t[G + 1])
            pz2a_t[G] = ps.tile([128, BT], F32, tag="pz2a", name="pz2a", bufs=2)
            z2(G, 0, h1a_t[G], pz2a_t[G])
            pz2b_t[G] = ps.tile([128, BT], F32, tag="pz2b", name="pz2b")
            z2(G, 1, h1b_t[G], pz2b_t[G])
            if G + 1 < NGRP:
                alloc_h1(G + 1)
                h1drain(G + 1, 0, pz1a_t[G + 1], h1a_t[G + 1])
                h1drain(G + 1, 1, pz1b_t[G + 1], h1b_t[G + 1])
            h2a_t[G] = h2_pool.tile([128, BT], BF16, tag="h2a", name="h2a")
            h2drain(G, 0, pz2a_t[G], h2a_t[G])
            h2b_t[G] = h2_pool.tile([128, BT], BF16, tag="h2b", name="h2b")
            h2drain(G, 1, pz2b_t[G], h2b_t[G])

        z3quad(NGRP + 1)  # (NGRP-2, NGRP-1)

    nc.compile()
    return nc


def _prep_shared(W1, b1, W2, b2, W3):
    import ml_dtypes

    bf = ml_dtypes.bfloat16
    w1q = np.ascontiguousarray(W1.reshape(4, 32 * S)).astype(bf)
    b1q = np.ascontiguousarray(b1.reshape(4, 32 * S)).astype(bf)
    w2t = np.ascontiguousarray(W2.transpose(1, 0, 2).reshape(S, F * H1)).astype(bf)
    b2p = np.empty((2 * H1, F // 2), np.float32)
    w3p = np.empty((2 * H1, F // 2), np.float32)
    W3f = W3.reshape(F, H1)
    for j in range(F // 2):
        b2p[:H1, j] = b2[j]
        b2p[H1:, j] = b2[j + 64]
        w3p[:H1, j] = W3f[j]
        w3p[H1:, j] = W3f[j + 64]
    return {
        "w1q": w1q,
        "b1q": b1q,
        "w2t": w2t,
        "b2p": b2p,
        "w3p": w3p.astype(bf),
        "ones": np.ones((1, 32 * BLOC), bf),
    }


def _prep_core_inputs(xc, shared):
    import ml_dtypes

    m = dict(shared)
    # xg[i, g*BLOC + b] = x[b, 32i+g]
    m["xg"] = (
        np.ascontiguousarray(xc.T.reshape(4, 32 * BLOC)).astype(ml_dtypes.bfloat16)
    )
    return m


def kernel(x, W1, b1, W2, b2, W3, b3, bias, _trace=False):
    x = np.asarray(x, np.float32)
    W1 = np.asarray(W1, np.float32)
    b1 = np.asarray(b1, np.float32)
    W2 = np.asarray(W2, np.float32)
    b2 = np.asarray(b2, np.float32)
    W3 = np.asarray(W3, np.float32)
    b3 = np.asarray(b3, np.float32)
    bias = np.asarray(bias, np.float32)

    if "nc" not in _CACHE:
        _CACHE["nc"] = _build()
    nc = _CACHE["nc"]

    shared = _prep_shared(W1, b1, W2, b2, W3)
    in_maps = [
        _prep_core_inputs(x[c * BLOC : (c + 1) * BLOC], shared) for c in range(NCORES)
    ]

    res = run_bass_kernel_spmd(nc, in_maps, core_ids=list(range(NCORES)), trace=_trace)
    _CACHE["last_result"] = res

    const = float(b3.sum()) + float(bias.reshape(-1)[0])
    parts = []
    for c in range(NCORES):
        o = res.results[c]["out"]  # [NBT*4, BT]: pout rows 0/32/64/96 per chunk
        parts.append(o.reshape(NBT, 4, BT).sum(axis=1).reshape(BLOC))
    out = np.concatenate(parts) + const
    return out.reshape(B, 1).astype(np.float32)


# revision 22
# speedup vs baseline: 1.2433x; 1.0164x over previous
"""NeuralAdditiveModel TRN2 kernel.

out[b] = sum_f ( relu(relu(x[b,f]*W1[f,:]+b1[f,:]) @ W2[f] + b2[f]) @ W3[f] + b3[f] ) + bias

Sharding: data-parallel over batch, 8 cores x 1024 rows. No collectives.

Per-core dataflow: 64 groups, each 4 features x 512-batch chunk, software
pipelined. The PE on this part streams at ~1.2GHz, so phase count per group
is what matters; it runs 3.5 phases of ~512 cycles per group:
  z1 : four K=2 matmuls row-tiled on all 4 strips (one phase) -> pz1a/pz1b
  z2 : two col-tiled M=64 matmuls per slot (two phases)       -> pz2a/pz2b
  z3 : every other group, FOUR M=1 matmuls on col strips 0..3, concurrent
       (one phase per two groups) -> pout rows 0/32/64/96
Relu drains are fixed-assigned: DVE h1a+h2b, ACT h1b+h2a (~balanced for the
1.2 vs 0.96 GHz engine rates).

Group G=(bt,g) covers feats {g, g+64} (slot a) and {g+32, g+96} (slot b);
issue order per G: z1(G+1), z2(G), h1(G+1), z3quad(G-2,G-1), h2(G).
PSUM: pz1 4 banks + pz2 2 + pout 1 = 7 of 8.
"""

import sys
from contextlib import ExitStack

import numpy as np

sys.path.insert(0, "/opt/trn_rl_repo")

import concourse.bass as bass  # noqa: E402
import concourse.tile as tile  # noqa: E402
from concourse import bacc, mybir  # noqa: E402
from concourse.bass_utils import run_bass_kernel_spmd  # noqa: E402

B, F, S, H1 = 8192, 128, 128, 64
NCORES = 8
BLOC = B // NCORES   # 1024 rows per core
BT = 512             # batch chunk (PSUM bank width in fp32)
NBT = BLOC // BT     # 2
NG = 32              # feature groups per chunk
NGRP = NBT * NG      # 64 pipeline groups
F32 = mybir.dt.float32
BF16 = mybir.dt.bfloat16

_CACHE = {}


def _build():
    nc = bacc.Bacc(
        "TRN2",
        target_bir_lowering=False,
        debug=False,
        enable_asserts=False,
        num_devices=NCORES,
    )

    xg_d = nc.dram_tensor("xg", [4, 32 * BLOC], BF16, kind="ExternalInput").ap()
    ones_d = nc.dram_tensor("ones", [1, 32 * BLOC], BF16, kind="ExternalInput").ap()
    w1q_d = nc.dram_tensor("w1q", [4, 32 * S], BF16, kind="ExternalInput").ap()
    b1q_d = nc.dram_tensor("b1q", [4, 32 * S], BF16, kind="ExternalInput").ap()
    w2t_d = nc.dram_tensor("w2t", [S, F * H1], BF16, kind="ExternalInput").ap()
    b2p_d = nc.dram_tensor("b2p", [2 * H1, F // 2], F32, kind="ExternalInput").ap()
    w3p_d = nc.dram_tensor("w3p", [2 * H1, F // 2], BF16, kind="ExternalInput").ap()
    out_d = nc.dram_tensor("out", [NBT * 4, BT], F32, kind="ExternalOutput").ap()

    Relu = mybir.ActivationFunctionType.Relu
    Copy = mybir.ActivationFunctionType.Copy

    with tile.TileContext(nc) as tc, ExitStack() as ctx:
        singles = ctx.enter_context(tc.tile_pool(name="singles", bufs=1))
        h1_pool = ctx.enter_context(tc.tile_pool(name="h1p", bufs=6))
        h2_pool = ctx.enter_context(tc.tile_pool(name="h2p", bufs=8))
        ps = ctx.enter_context(tc.tile_pool(name="ps", bufs=1, space="PSUM"))

        # Persistent SBUF tensors
        xaug = singles.tile([128, 32 * BLOC], BF16)  # x rows (32i) + ones (32i+1)
        w1b1 = singles.tile([128, 32 * S], BF16)     # W1 rows (32i) + b1 (32i+1)
        w2sb = singles.tile([S, F * H1], BF16)       # W2, s-major
        b2p = singles.tile([2 * H1, F // 2], F32)    # paired bias columns
        w3p = singles.tile([2 * H1, F // 2], BF16)   # paired W3 columns

        # Setup DMAs spread across the 3 DMA-capable queues, ordered by first
        # use: all 16 z1 rows first (single-partition rows are the slow
        # transfers), then the w2 quarters, then small bias tables.
        FQ = F // 4 * H1  # w2t column quarter
        qs = (nc.sync, nc.scalar, nc.gpsimd)

        for i in range(4):
            qs[i % 3].dma_start(
                out=w1b1[32 * i : 32 * i + 1, :], in_=w1q_d[i : i + 1, :]
            )
            qs[(i + 1) % 3].dma_start(
                out=w1b1[32 * i + 1 : 32 * i + 2, :], in_=b1q_d[i : i + 1, :]
            )
        CW = 32 * BLOC // 4  # x/ones row quarter: 8 feature-blocks
        for c in range(4):
            cl, ch = c * CW, (c + 1) * CW
            for i in range(4):
                q = (i + c) % 3
                qs[q].dma_start(
                    out=xaug[32 * i : 32 * i + 1, cl:ch], in_=xg_d[i : i + 1, cl:ch]
                )
                qs[(q + 1) % 3].dma_start(
                    out=xaug[32 * i + 1 : 32 * i + 2, cl:ch], in_=ones_d[0:1, cl:ch]
                )
            if c == 0:
                # w2 quarters needed by the first z2s go right after chunk 0
                qs[0].dma_start(out=w2sb[:, 0:FQ], in_=w2t_d[:, 0:FQ])
                qs[1].dma_start(
                    out=w2sb[:, 2 * FQ : 3 * FQ], in_=w2t_d[:, 2 * FQ : 3 * FQ]
                )
                qs[2].dma_start(out=b2p, in_=b2p_d)
                qs[2].dma_start(out=w3p, in_=w3p_d)
            if c == 1:
                qs[0].dma_start(out=w2sb[:, FQ : 2 * FQ], in_=w2t_d[:, FQ : 2 * FQ])
                qs[1].dma_start(out=w2sb[:, 3 * FQ :], in_=w2t_d[:, 3 * FQ :])

        def grp(G):  # group -> (bt, g)
            return G // NG, G % NG

        def z1(G, pza, pzb):
            bt, g = grp(G)
            # strips q0,q32,q64,q96 <-> feats g, g+32, g+64, g+96; 4 banks
            for i, pz, half in ((0, pza, 0), (1, pzb, 0), (2, pza, 1), (3, pzb, 1)):
                r = 32 * i
                nc.tensor.matmul(
                    out=pz[:, half * BT : (half + 1) * BT],
                    lhsT=w1b1[r : r + 2, g * S : (g + 1) * S],
                    rhs=xaug[r : r + 2, g * BLOC + bt * BT : g * BLOC + (bt + 1) * BT],
                    start=True,
                    stop=True,
                    tile_position=(r, 0),
                )

        def z2(G, sub, h1sb, pz2):
            _, g = grp(G)
            j = g + 32 * sub
            for half, f in enumerate((j, j + 64)):
                nc.tensor.matmul(
                    out=pz2[64 * half : 64 * half + 64, :],
                    lhsT=w2sb[:, f * H1 : (f + 1) * H1],
                    rhs=h1sb[:, half * BT : (half + 1) * BT],
                    start=True,
                    stop=True,
                )

        def h1drain(G, sub, pz, h1sb):
            if sub == 0:
                nc.vector.tensor_scalar_max(h1sb, pz, 0.0)
            else:
                nc.scalar.activation(h1sb, pz, Relu)

        def h2drain(G, sub, pz2, h2sb):
            _, g = grp(G)
            j = g + 32 * sub
            if sub == 0:
                nc.scalar.activation(h2sb, pz2, Relu, bias=b2p[:, j : j + 1])
            else:
                nc.vector.tensor_scalar(
                    h2sb,
                    pz2,
                    b2p[:, j : j + 1],
                    0.0,
                    mybir.AluOpType.add,
                    mybir.AluOpType.max,
                )

        def z3(q, sub, h2sb, pout):
            bt, g = grp(q)
            j = g + 32 * sub
            row = 32 * (2 * (q % 2) + sub)  # col strips 0..3 across the quad
            nc.tensor.matmul(
                out=pout[row : row + 1, :],
                lhsT=w3p[:, j : j + 1],
                rhs=h2sb,
                start=(g <= 1),
                stop=(g >= NG - 2),
                skip_group_check=True,
                tile_position=(0, row),
            )

        def pout_flush(bt, pout):
            srow = h2_pool.tile([128, BT], F32, tag="srow", name="srow")
            nc.scalar.activation(srow[0:97, :], pout[0:97, :], Copy)
            srow_g = srow.rearrange("(i q) c -> i q c", q=32)
            nc.sync.dma_start(out=out_d[4 * bt : 4 * bt + 4, :], in_=srow_g[:, 0, :])

        pz1a_t = [None] * NGRP
        pz1b_t = [None] * NGRP
        h1a_t = [None] * NGRP
        h1b_t = [None] * NGRP
        pz2a_t = [None] * NGRP
        pz2b_t = [None] * NGRP
        h2a_t = [None] * NGRP
        h2b_t = [None] * NGRP
        pout_t = [None] * NBT

        def alloc_z1(G):
            pz1a_t[G] = ps.tile([128, 2 * BT], F32, tag="pz1a", name="pz1a")
            pz1b_t[G] = ps.tile([128, 2 * BT], F32, tag="pz1b", name="pz1b")

        def alloc_h1(G):
            h1a_t[G] = h1_pool.tile([128, 2 * BT], BF16, tag="h1a", name="h1a")
            h1b_t[G] = h1_pool.tile([128, 2 * BT], BF16, tag="h1b", name="h1b")

        def z3quad(G):
            # z3 for groups G-3, G-2 as four concurrent col-tiled matmuls
            # (1.5+ periods stale, so the quad never blocks the PE queue)
            for q in (G - 3, G - 2):
                bt, g = grp(q)
                if g == 0:
                    pout_t[bt] = ps.tile([128, BT], F32, tag="pout", name="pout")
                z3(q, 0, h2a_t[q], pout_t[bt])
                z3(q, 1, h2b_t[q], pout_t[bt])
                if g == NG - 1:
                    pout_flush(bt, pout_t[bt])

        alloc_z1(0)
        z1(0, pz1a_t[0], pz1b_t[0])
        alloc_h1(0)
        h1drain(0, 0, pz1a_t[0], h1a_t[0])
        h1drain(0, 1, pz1b_t[0], h1b_t[0])

        for G in range(NGRP):
            if G >= 3 and G % 2 == 1:
                z3quad(G)
            if G + 1 < NGRP:
                alloc_z1(G + 1)
                z1(G + 1, pz1a_t[G + 1], pz1b_t[G + 1])
            pz2a_t[G] = ps.tile([128, BT], F32, tag="pz2a", name="pz2a", bufs=2)
            z2(G, 0, h1a_t[G], pz2a_t[G])
            pz2b_t[G] = ps.tile([128, BT], F32, tag="pz2b", name="pz2b")
            z2(G, 1, h1b_t[G], pz2b_t[G])
            if G + 1 < NGRP:
                alloc_h1(G + 1)
                h1drain(G + 1, 0, pz1a_t[G + 1], h1a_t[G + 1])
                h1drain(G + 1, 1, pz1b_t[G + 1], h1b_t[G + 1])
            h2a_t[G] = h2_pool.tile([128, BT], BF16, tag="h2a", name="h2a")
            h2drain(G, 0, pz2a_t[G], h2a_t[G])
            h2b_t[G] = h2_pool.tile([128, BT], BF16, tag="h2b", name="h2b")
            h2drain(G, 1, pz2b_t[G], h2b_t[G])

        z3quad(NGRP + 1)  # (NGRP-2, NGRP-1)

    nc.compile()
    return nc


def _prep_shared(W1, b1, W2, b2, W3):
    import ml_dtypes

    bf = ml_dtypes.bfloat16
    w1q = np.ascontiguousarray(W1.reshape(4, 32 * S)).astype(bf)
    b1q = np.ascontiguousarray(b1.reshape(4, 32 * S)).astype(bf)
    w2t = np.ascontiguousarray(W2.transpose(1, 0, 2).reshape(S, F * H1)).astype(bf)
    b2p = np.empty((2 * H1, F // 2), np.float32)
    w3p = np.empty((2 * H1, F // 2), np.float32)
    W3f = W3.reshape(F, H1)
    for j in range(F // 2):
        b2p[:H1, j] = b2[j]
        b2p[H1:, j] = b2[j + 64]
        w3p[:H1, j] = W3f[j]
        w3p[H1:, j] = W3f[j + 64]
    return {
        "w1q": w1q,
        "b1q": b1q,
        "w2t": w2t,
        "b2p": b2p,
        "w3p": w3p.astype(bf),
        "ones": np.ones((1, 32 * BLOC), bf),
    }


def _prep_core_inputs(xc, shared):
    import ml_dtypes

    m = dict(shared)
    # xg[i, g*BLOC + b] = x[b, 32i+g]
    m["xg"] = (
        np.ascontiguousarray(xc.T.reshape(4, 32 * BLOC)).astype(ml_dtypes.bfloat16)
    )
    return m


def kernel(x, W1, b1, W2, b2, W3, b3, bias, _trace=False):
    x = np.asarray(x, np.float32)
    W1 = np.asarray(W1, np.float32)
    b1 = np.asarray(b1, np.float32)
    W2 = np.asarray(W2, np.float32)
    b2 = np.asarray(b2, np.float32)
    W3 = np.asarray(W3, np.float32)
    b3 = np.asarray(b3, np.float32)
    bias = np.asarray(bias, np.float32)

    if "nc" not in _CACHE:
        _CACHE["nc"] = _build()
    nc = _CACHE["nc"]

    shared = _prep_shared(W1, b1, W2, b2, W3)
    in_maps = [
        _prep_core_inputs(x[c * BLOC : (c + 1) * BLOC], shared) for c in range(NCORES)
    ]

    res = run_bass_kernel_spmd(nc, in_maps, core_ids=list(range(NCORES)), trace=_trace)
    _CACHE["last_result"] = res

    const = float(b3.sum()) + float(bias.reshape(-1)[0])
    parts = []
    for c in range(NCORES):
        o = res.results[c]["out"]  # [NBT*4, BT]: pout rows 0/32/64/96 per chunk
        parts.append(o.reshape(NBT, 4, BT).sum(axis=1).reshape(BLOC))
    out = np.concatenate(parts) + const
    return out.reshape(B, 1).astype(np.float32)
